# revision 1
# baseline (speedup 1.0000x reference)
"""Trainium2 Bass kernel for nn_CrossAttention_13537736917149.

Sharding: data-parallel over the B=8 scene axis, one scene per NeuronCore.
The host gathers each scene's points (xF[perm[b]]), transposes + fp8-quantizes
them for the on-device matmul layout, and scatters per-scene outputs back.

Device math per core (scene b, NPB=8192 points, K=256 ctx tokens,
H=8 heads x D=64, CH=256 channels):

  Prologue (once):
    kT_h = Wk_h^T @ ctx^T                   [64, K]  per head
    WK_h = Wq_h @ kT_h                      [CH, K]  per head  -> fp8
      (folds the whole q-projection into the score matrix:
       scores_h = WK_h^T @ x^T, contraction CH=256 = 2x128 tiles
       -> fp8 DoubleRow matmuls at 0.5 cycles/row)
    v8_h = ctx @ Wv_h                       [K, D]   fp8

  Chunk loop (16 chunks of 512 q), heads processed in pairs (2j, 2j+1):
    scores: 2 DR matmuls/head -> psum [128, 2x512] (2 banks)
    expT:   one ACT exp [128,1024] -> fp8 sbuf per head
    oU:     1 DR matmul/head -> pair psum X [oU_2j(rows 0-63); oU_2j+1]
    den:    1 DR matmul/head (ones stationary) -> pair psum Dn same split
    norm:   DVE copy Dn->sbuf + one DVE divide per PAIR
            (psum in0 + sbuf in1 at equal base partition 0 is legal;
             gpsimd can't touch psum and sbuf+sbuf needs equal bases)
    oT:     bf16 [128, 4, 512], head pairs on partitions
    out proj: 4 bf16 matmuls per 128-q tile + DVE residual add (x+b_out)
"""

import ml_dtypes
import numpy as np

import concourse.bass as bass
import concourse.mybir as mybir
import concourse.tile as tile
from concourse import bacc
from concourse.bass import ds, ts
from concourse.bass_utils import run_bass_kernel_spmd

# Problem dims (hardcoded per harness contract)
N, CH = 65536, 256
B, K, CTX = 8, 256, 768
H, D = 8, 64
HD = H * D  # 512
NPB = N // B  # 8192

F32 = mybir.dt.float32
F32R = mybir.dt.float32r
BF16 = mybir.dt.bfloat16
FP8 = mybir.dt.float8e4
Exp = mybir.ActivationFunctionType.Exp
DR = mybir.MatmulPerfMode.DoubleRow
NP_FP8 = mybir.dt.np(FP8)

SCALE = float(D) ** -0.5  # 0.125


def build_kernel(npb=NPB, chunk=512, n_cores=8, repeat=1):
    nchunks = npb // chunk
    nqt = chunk // 128  # 128-row q tiles per chunk

    nc = bacc.Bacc(
        "TRN2", target_bir_lowering=False, debug=False, num_devices=n_cores
    )

    x8_d = nc.dram_tensor("x8", [CH, npb], FP8, kind="ExternalInput")
    xb_d = nc.dram_tensor("xb", [npb, CH], BF16, kind="ExternalInput")
    ctxT_d = nc.dram_tensor("ctxT", [CTX, K], F32R, kind="ExternalInput")
    wqT_d = nc.dram_tensor("WqT", [HD, CH], F32R, kind="ExternalInput")
    wk_d = nc.dram_tensor("Wk", [CTX, HD], F32R, kind="ExternalInput")
    wv_d = nc.dram_tensor("Wv", [CTX, HD], F32R, kind="ExternalInput")
    wo_d = nc.dram_tensor("Wout", [HD, CH], BF16, kind="ExternalInput")
    ones_d = nc.dram_tensor("ones8", [128, 2, D], FP8, kind="ExternalInput")
    y_d = nc.dram_tensor("y", [npb, CH], F32, kind="ExternalOutput")

    # DRAM views tiled to 128 partitions
    x8_v = x8_d.ap().rearrange("(co p) n -> p co n", p=128)  # [128, 2, npb]
    xb_v = xb_d.ap().rearrange("(c q p) ch -> p c q ch", p=128, q=nqt)
    y_v = y_d.ap().rearrange("(c q p) ch -> p c q ch", p=128, q=nqt)
    ctxT_v = ctxT_d.ap().rearrange("(co p) k -> p co k", p=128)  # [128, 6, 256]
    wqT_v = wqT_d.ap().rearrange("(co p) ch -> p co ch", p=128)  # [128, 4, 256]
    wk_v = wk_d.ap().rearrange("(co p) hd -> p co hd", p=128)  # [128, 6, 512]
    wv_v = wv_d.ap().rearrange("(co p) hd -> p co hd", p=128)  # [128, 6, 512]
    wo_v = wo_d.ap().rearrange("(co p) ch -> p co ch", p=128)  # [128, 4, 256]

    with tile.TileContext(nc) as tc:
        with (
            tc.tile_pool(name="const", bufs=1) as p_const,
            tc.tile_pool(name="xin", bufs=3) as p_x,
            tc.tile_pool(name="xbp", bufs=2) as p_xb,
            tc.tile_pool(name="exp", bufs=4) as p_exp,
            tc.tile_pool(name="den", bufs=2) as p_den,
            tc.tile_pool(name="o", bufs=2) as p_o,
            tc.tile_pool(name="y", bufs=4) as p_y,
            tc.tile_pool(name="ps_s", bufs=2, space="PSUM") as p_ps_s,
            tc.tile_pool(name="ps_x", bufs=2, space="PSUM") as p_ps_x,
            tc.tile_pool(name="ps_d", bufs=1, space="PSUM") as p_ps_d,
            tc.tile_pool(name="ps_y", bufs=1, space="PSUM") as p_ps_y,
        ):
            # ---- constants / weights ----
            wqT_sb = p_const.tile([128, 4, CH], F32R)
            nc.sync.dma_start(wqT_sb[:], wqT_v)
            wk_sb = p_const.tile([128, 6, HD], F32R)
            nc.sync.dma_start(wk_sb[:], wk_v)
            wv_sb = p_const.tile([128, 6, HD], F32R)
            nc.sync.dma_start(wv_sb[:], wv_v)
            wo_sb = p_const.tile([128, 4, CH], BF16)
            nc.sync.dma_start(wo_sb[:], wo_v)
            ctxT_sb = p_const.tile([128, 6, K], F32R)
            nc.sync.dma_start(ctxT_sb[:], ctxT_v)
            ones_sb = p_const.tile([128, 2, D], FP8)
            nc.sync.dma_start(ones_sb[:], ones_d.ap())

            # main-loop operand tiles (filled in prologue)
            wk8_sb = p_const.tile([128, 2, H, 2, 128], FP8)  # WK fp8
            v8_sb = p_const.tile([128, 2, H, D], FP8)  # v fp8

            if True:
                p_pro = p_ps_s  # reuse main-loop psum pool for prologue
                # kT_h: [hd_p, hd_o, tok], head h at partitions (h%2)*64..
                # of free-tile h//2 (same layout as wqT view)
                kT_sb = p_const.tile([128, 4, K], F32R)
                for hdt in range(4):
                    ps_k = p_pro.tile([128, K], F32, tag="s", name=f"ps_k{hdt}")
                    for ct in range(6):
                        nc.tensor.matmul(
                            ps_k[:],
                            wk_sb[:, ct, ts(hdt, 128)],
                            ctxT_sb[:, ct, :],
                            start=(ct == 0),
                            stop=(ct == 5),
                        )
                    nc.scalar.copy(kT_sb[:, hdt, :], ps_k[:])

                # WK_h[ch, k] = sum_d Wq[ch, d_h] * kT_h[d, k]
                for h in range(H):
                    r0, j = (h % 2) * 64, h // 2
                    for ct in range(2):
                        ps_wk = p_pro.tile(
                            [128, K], F32, tag="s", name=f"ps_wk{h}_{ct}"
                        )
                        nc.tensor.matmul(
                            ps_wk[:],
                            wqT_sb[ds(r0, 64), j, ts(ct, 128)],
                            kT_sb[ds(r0, 64), j, :],
                            start=True,
                            stop=True,
                        )
                        nc.vector.tensor_copy(wk8_sb[:, ct, h, :, :], ps_wk[:])

                # v8: [tok_p, kt, h, D] fp8
                for tt in range(2):
                    ps_v = p_pro.tile([128, HD], F32, tag="s", name=f"ps_v{tt}")
                    for ct in range(6):
                        nc.tensor.matmul(
                            ps_v[:],
                            ctxT_sb[:, ct, ts(tt, 128)],
                            wv_sb[:, ct, :],
                            start=(ct == 0),
                            stop=(ct == 5),
                        )
                    nc.vector.tensor_copy(v8_sb[:, tt, :, :], ps_v[:])

            # ---- main loop over q chunks ----
            import contextlib

            rep_cm = (
                tc.For_i(0, repeat, 1) if repeat > 1 else contextlib.nullcontext()
            )
            with rep_cm:
                main_body(
                    nc, tc, nchunks, chunk, nqt,
                    x8_v, xb_v, y_v, wk8_sb, v8_sb, ones_sb, wo_sb,
                    p_x, p_xb, p_exp, p_den, p_o, p_y,
                    p_ps_s, p_ps_x, p_ps_d, p_ps_y,
                )

    nc.compile()
    return nc


def main_body(
    nc, tc, nchunks, chunk, nqt,
    x8_v, xb_v, y_v, wk8_sb, v8_sb, ones_sb, wo_sb,
    p_x, p_xb, p_exp, p_den, p_o, p_y,
    p_ps_s, p_ps_x, p_ps_d, p_ps_y,
):
    def emit_outproj(state):
        pc, oT_p, xb_p = state
        for qt in range(nqt):
            ps_y = p_ps_y.tile([128, CH], F32, tag="psy", name=f"ps_y_{pc}_{qt}")
            for j in range(4):
                nc.tensor.matmul(
                    ps_y[:],
                    oT_p[:, j, ts(qt, 128)],
                    wo_sb[:, j, :],
                    start=(j == 0),
                    stop=(j == 3),
                )
            y_t = p_y.tile([128, CH], F32, tag="y", name=f"y_{pc}_{qt}")
            nc.vector.tensor_add(out=y_t[:], in0=ps_y[:], in1=xb_p[:, qt, :])
            nc.sync.dma_start(y_v[:, pc, qt], y_t[:])

    def emit_scores(c, h, xt_t):
        # scores: 2 fp8 DoubleRow matmuls into a 2-bank psum tile;
        # one wide exp -> fp8 expT
        ps_s = p_ps_s.tile([128, 2, chunk], F32, tag="s", name=f"ps_s_{c}_{h}")
        for kt in range(2):
            nc.tensor.matmul(
                ps_s[:, kt, :],
                wk8_sb[:, :, h, kt, :],
                xt_t[:],
                start=True,
                stop=True,
                perf_mode=DR,
            )
        e_t = p_exp.tile([128, 2, chunk], FP8, tag="e", name=f"e_{c}_{h}")
        nc.scalar.activation(e_t[:], ps_s[:], Exp, scale=SCALE)
        return e_t

    def emit_av(c, j, e_pair, ps_x_t, ps_d_t):
        # oU and den matmuls for the head pair (2j, 2j+1): even head on
        # psum rows 0-63 via cheap DoubleRow matmuls (DR outputs must sit
        # at partition 0), odd head on rows 64-127 via 2 accumulating
        # non-DR fp8 matmuls
        h0, h1 = 2 * j, 2 * j + 1
        nc.tensor.matmul(
            ps_x_t[ds(0, 64), :], v8_sb[:, :, h0, :], e_pair[0][:],
            start=True, stop=True, perf_mode=DR,
        )
        nc.tensor.matmul(
            ps_d_t[ds(0, 64), :], ones_sb[:], e_pair[0][:],
            start=True, stop=True, perf_mode=DR,
        )
        for kt in range(2):
            nc.tensor.matmul(
                ps_x_t[ds(64, 64), :], v8_sb[:, kt, h1, :],
                e_pair[1][:, kt, :], start=(kt == 0), stop=(kt == 1),
            )
        for kt in range(2):
            nc.tensor.matmul(
                ps_d_t[ds(64, 64), :], ones_sb[:, kt, :],
                e_pair[1][:, kt, :], start=(kt == 0), stop=(kt == 1),
            )

    def emit_norm(c, j, ps_x_t, ps_d_t, oT_t):
        den_t = p_den.tile([128, chunk], F32, tag="d", name=f"den_{c}_{j}")
        nc.vector.reciprocal(den_t[:], ps_d_t[:])
        nc.vector.tensor_mul(
            out=oT_t[:, j, :],
            in0=ps_x_t[:],
            in1=den_t[:],
        )

    prev = None
    for c in range(nchunks):
        xt_t = p_x.tile([128, 2, chunk], FP8, tag="xt", name=f"xt_{c}")
        nc.sync.dma_start(xt_t[:], x8_v[:, :, ds(c * chunk, chunk)])
        xb_t = p_xb.tile([128, nqt, CH], BF16, tag="xb", name=f"xb_{c}")
        nc.sync.dma_start(xb_t[:], xb_v[:, c])

        oT_t = p_o.tile([128, 4, chunk], BF16, tag="o", name=f"oT_{c}")
        e_prev = None
        for j in range(4):
            e_pair = [emit_scores(c, 2 * j, xt_t), emit_scores(c, 2 * j + 1, xt_t)]
            if e_prev is not None:
                ps_x_t = p_ps_x.tile(
                    [128, chunk], F32, tag="x", name=f"ps_x_{c}_{j-1}"
                )
                ps_d_t = p_ps_d.tile(
                    [128, chunk], F32, tag="dn", name=f"ps_d_{c}_{j-1}"
                )
                emit_av(c, j - 1, e_prev, ps_x_t, ps_d_t)
                emit_norm(c, j - 1, ps_x_t, ps_d_t, oT_t)
            e_prev = e_pair
            if j == 1 and prev is not None:
                emit_outproj(prev)
        ps_x_t = p_ps_x.tile([128, chunk], F32, tag="x", name=f"ps_x_{c}_3")
        ps_d_t = p_ps_d.tile([128, chunk], F32, tag="dn", name=f"ps_d_{c}_3")
        emit_av(c, 3, e_prev, ps_x_t, ps_d_t)
        emit_norm(c, 3, ps_x_t, ps_d_t, oT_t)
        prev = (c, oT_t, xb_t)
    emit_outproj(prev)


_NC_CACHE = {}


def _get_nc(npb=NPB, chunk=512, n_cores=8, repeat=1):
    key = (npb, chunk, n_cores, repeat)
    if key not in _NC_CACHE:
        _NC_CACHE[key] = build_kernel(npb, chunk, n_cores, repeat)
    return _NC_CACHE[key]


def prep_in_maps(xF, context, perm, Wq, Wk, Wv, Wout, b_out):
    """Host-side shard prep shared by kernel() and test harnesses."""
    xF = np.asarray(xF, dtype=np.float32)
    context = np.asarray(context, dtype=np.float32)
    perm = np.asarray(perm, dtype=np.int32).reshape(B, NPB)
    Wq = np.ascontiguousarray(np.asarray(Wq, dtype=np.float32))
    Wk = np.ascontiguousarray(np.asarray(Wk, dtype=np.float32))
    Wv = np.ascontiguousarray(np.asarray(Wv, dtype=np.float32))
    Wout = np.ascontiguousarray(np.asarray(Wout, dtype=np.float32))
    b_out = np.asarray(b_out, dtype=np.float32)

    wqT = np.ascontiguousarray(Wq.T)  # [HD, CH]
    wo16 = Wout.astype(ml_dtypes.bfloat16)
    ones8 = np.ones((128, 2, D), dtype=NP_FP8)

    in_maps = []
    for b in range(B):
        xg = xF[perm[b]]  # [NPB, CH]
        in_maps.append(
            {
                "x8": np.ascontiguousarray(xg.T).astype(NP_FP8),
                "xb": (xg + b_out[None, :]).astype(ml_dtypes.bfloat16),
                "ctxT": np.ascontiguousarray(context[b].T),
                "WqT": wqT,
                "Wk": Wk,
                "Wv": Wv,
                "Wout": wo16,
                "ones8": ones8,
            }
        )
    return in_maps, perm


def kernel(xF, context, perm, Wq, Wk, Wv, Wout, b_out, _trace=False):
    in_maps, perm_flat = prep_in_maps(
        xF, context, perm, Wq, Wk, Wv, Wout, b_out
    )
    nc = _get_nc()
    res = run_bass_kernel_spmd(
        nc, in_maps, core_ids=list(range(B)), trace=_trace
    )
    out = np.empty((N, CH), dtype=np.float32)
    for b in range(B):
        out[perm_flat[b]] = res.results[b]["y"]
    if _trace:
        kernel.last_exec_time_ns = res.exec_time_ns
        kernel.last_results = res
    return out

